# revision 38
# baseline (speedup 1.0000x reference)
"""BiDAF attention kernel for 8 Trainium2 NeuronCores.

Data-parallel over batch (B=32 -> 4 per core). Per batch, on-chip:
  sT[j,i] = (q*cqw) @ c^T + s0[i] + s1[j] + bias   (bf16 matmuls, fp32 accum;
  s0 comes free as rows 64/96 of the same matmul via cwgt hi/lo columns, then
  a rank-1 fp32 matmul broadcasts it across rows)
  E = exp(sT)  (one exp serves both softmaxes; s1+bias fused via act bias)
  a2T = E / rowsum(E);  a1 normalization deferred: 1/colsum(E) applied to
  output rows of downstream matmuls.
  a = a1 @ q; b = a1 @ (a2^T @ c); out = [c, a, c*a, c*b]
q-side prep (casts, qs, s1, q^T) is batch-pair-packed to fill 128 partitions.
DMA: input loads dispatched first on the sync queue, c-passthrough stores on
the scalar queue, m-tile stores stream on sync behind compute.
"""

import sys

if "/opt/trn_rl_repo" not in sys.path:
    sys.path.insert(0, "/opt/trn_rl_repo")

from contextlib import ExitStack

import numpy as np

import concourse.bacc as bacc
import concourse.bass as bass
import concourse.mybir as mybir
from concourse.bass import ts
from concourse.bass_utils import run_bass_kernel_spmd
from concourse.masks import make_identity
from concourse.tile import TileContext

N_CORES = 8
B, Lc, Lq, H = 32, 512, 64, 512
BPC = B // N_CORES  # batches per core
F32 = mybir.dt.float32
BF16 = mybir.dt.bfloat16
MULT = mybir.AluOpType.mult

_CACHE = {}


def _build_program():
    nc = bacc.Bacc("TRN2", target_bir_lowering=False, debug=False, num_devices=N_CORES)
    c_h = nc.dram_tensor("c", [BPC, Lc, H], F32, kind="ExternalInput")
    q_h = nc.dram_tensor("q", [BPC, Lq, H], F32, kind="ExternalInput")
    cqw_h = nc.dram_tensor("cqw", [H], F32, kind="ExternalInput")
    cwgt_h = nc.dram_tensor("cwgt", [H], F32, kind="ExternalInput")
    qwgt_h = nc.dram_tensor("qwgt", [H], F32, kind="ExternalInput")
    bias_h = nc.dram_tensor("bias", [1], F32, kind="ExternalInput")
    out_h = nc.dram_tensor("out", [BPC, Lc, 3 * H], F32, kind="ExternalOutput")

    c_ap = c_h.ap()
    q_ap = q_h.ap()
    out_ap = out_h.ap()

    exp_f = mybir.ActivationFunctionType.Exp
    ident_f = mybir.ActivationFunctionType.Identity
    copy_f = mybir.ActivationFunctionType.Copy

    with TileContext(nc) as tc, ExitStack() as ctx:
        const = ctx.enter_context(tc.tile_pool(name="const", bufs=1))
        cpool = ctx.enter_context(tc.tile_pool(name="cpool", bufs=4))
        cbpool = ctx.enter_context(tc.tile_pool(name="cbpool", bufs=3))
        ctpool = ctx.enter_context(tc.tile_pool(name="ctpool", bufs=3))
        qpool = ctx.enter_context(tc.tile_pool(name="qpool", bufs=2))
        spool = ctx.enter_context(tc.tile_pool(name="spool", bufs=3))
        lpool = ctx.enter_context(tc.tile_pool(name="lpool", bufs=3))
        epool = ctx.enter_context(tc.tile_pool(name="epool", bufs=3))
        btpool = ctx.enter_context(tc.tile_pool(name="btpool", bufs=2))
        opool = ctx.enter_context(tc.tile_pool(name="opool", bufs=4))
        ps_tr = ctx.enter_context(tc.tile_pool(name="ps_tr", bufs=1, space="PSUM"))
        ps_mm = ctx.enter_context(tc.tile_pool(name="ps_mm", bufs=3, space="PSUM"))
        ps_ab = ctx.enter_context(tc.tile_pool(name="ps_ab", bufs=2, space="PSUM"))

        NP = BPC // 2  # pairs per core

        # ---- input loads: dispatched first on the sync queue ----
        c_tiles = []
        q_pairs = []
        for p in range(NP):
            b0, b1 = 2 * p, 2 * p + 1
            q_t = qpool.tile([128, H], F32, name="q_sb")
            nc.sync.dma_start(out=q_t[0:Lq, :], in_=q_ap[b0])
            nc.sync.dma_start(out=q_t[Lq:128, :], in_=q_ap[b1])
            c0 = cpool.tile([128, 4, H], F32, name="c_sb")
            nc.sync.dma_start(out=c0, in_=c_ap[b0].rearrange("(j p) h -> p j h", p=128))
            c1 = cpool.tile([128, 4, H], F32, name="c_sb")
            nc.sync.dma_start(out=c1, in_=c_ap[b1].rearrange("(j p) h -> p j h", p=128))
            c_tiles += [c0, c1]
            q_pairs.append(q_t)

        # ---- constants (gpsimd queue; overlap the input loads) ----
        ident = const.tile([128, 128], BF16, name="ident")
        make_identity(nc, ident)
        cw_bc = const.tile([128, H], F32, name="cw_bc")  # cq_weight bcast over rows
        nc.gpsimd.dma_start(out=cw_bc, in_=bass.AP(tensor=cqw_h, offset=0, ap=[[0, 128], [1, H]]))
        qw_bc = const.tile([128, H], F32, name="qw_bc")  # q_weight bcast over rows
        nc.gpsimd.dma_start(out=qw_bc, in_=bass.AP(tensor=qwgt_h, offset=0, ap=[[0, 128], [1, H]]))
        cwgt_col = const.tile([128, 4], F32, name="cwgt_col")  # c_weight as 4 chunks
        nc.gpsimd.dma_start(out=cwgt_col, in_=bass.AP(tensor=cwgt_h, offset=0, ap=[[1, 128], [128, 4]]))
        cwgt_hi = const.tile([128, 4], BF16, name="cwgt_hi")
        nc.vector.tensor_copy(out=cwgt_hi, in_=cwgt_col)
        cwgt_res = const.tile([128, 4], F32, name="cwgt_res")
        nc.vector.tensor_sub(cwgt_res, cwgt_col, cwgt_hi)
        cwgt_lo = const.tile([128, 4], BF16, name="cwgt_lo")
        nc.vector.tensor_copy(out=cwgt_lo, in_=cwgt_res)
        bias_bc = const.tile([128, 1], F32, name="bias_bc")
        nc.gpsimd.dma_start(out=bias_bc, in_=bass.AP(tensor=bias_h, offset=0, ap=[[0, 128], [1, 1]]))
        ones_col = const.tile([128, 1], BF16, name="ones_col")
        nc.vector.memset(ones_col, 1.0)
        aug_f = const.tile([128, 97], F32, name="aug_f")
        nc.vector.memset(aug_f, 0.0)
        nc.vector.memset(aug_f[64:65, 0:64], 1.0)
        nc.vector.memset(aug_f[96:97, 0:64], 1.0)
        aug = const.tile([128, 97], mybir.dt.float32r, name="aug")  # s0 hi+lo add
        nc.vector.tensor_copy(out=aug, in_=aug_f)

        P = [dict() for _ in range(NP)]  # per-pair tile state
        S = [dict() for _ in range(BPC)]  # per-batch tile state

        def stage_PQ(p):
            """pair-level q-side prep: casts, qs, s1, qT transposes, lhsT."""
            q_sb = q_pairs[p]
            q_bf = qpool.tile([128, H], BF16, name="q_bf")
            nc.vector.tensor_copy(out=q_bf, in_=q_sb)
            # qs = q * cq_weight (bf16 out) ; s1 = (q @ q_weight) + bias
            qs_bf = qpool.tile([128, H], BF16, name="qs_bf")
            nc.vector.tensor_mul(qs_bf, q_sb, cw_bc)
            s1_scr = qpool.tile([128, H], F32, name="s1_scr")
            s1_raw = spool.tile([128, 1], F32, name="s1_raw")
            nc.gpsimd.tensor_mul(s1_scr, q_sb, qw_bc)
            nc.vector.tensor_reduce(
                out=s1_raw, in_=s1_scr, axis=mybir.AxisListType.X,
                op=mybir.AluOpType.add,
            )
            s1b = spool.tile([128, 1], F32, name="s1b")
            nc.scalar.activation(out=s1b, in_=s1_raw, func=ident_f, bias=bias_bc, scale=1.0)

            # qT chunks for the pair: pt_q[:, f, 0:64] = b0, [:, f, 64:128] = b1
            qsT = ps_tr.tile([128, 4, 128], BF16, name="qsT", tag="trq", bufs=1)
            for f in range(4):
                nc.tensor.transpose(qsT[:, f, :], qs_bf[:, ts(f, 128)], ident)
            # lhsT[b][f] = [ (qs_b chunk f)^T | cwgt_hi f | junk | cwgt_lo f ]
            # (hi lands in psum row 64, lo in row 96: engine reads need
            # 32-aligned base partitions; rows 65..95 are unused junk)
            for b in (2 * p, 2 * p + 1):
                off = (b % 2) * 64
                lhsT = lpool.tile([128, 4, 97], BF16, name="lhsT")
                # zero cols 65..95: psum rows 65..95 must be finite (the s0
                # aug matmul contracts over them with zero weights)
                nc.vector.memset(lhsT[:, :, 65:96], 0.0)
                nc.vector.tensor_copy(out=lhsT[:, :, 0:64], in_=qsT[:, :, off:off + 64])
                nc.vector.tensor_copy(out=lhsT[:, :, 64:65], in_=cwgt_hi.rearrange("p (f o) -> p f o", o=1))
                nc.vector.tensor_copy(out=lhsT[:, :, 96:97], in_=cwgt_lo.rearrange("p (f o) -> p f o", o=1))
                S[b]["lhsT"] = lhsT
            # odd batch's q half must sit at base partition 0 for the a-matmul
            # (PE requires lhsT/rhs at the same base partition); DMA it down.
            q_bf1 = qpool.tile([Lq, H], BF16, name="q_bf1")
            nc.scalar.dma_start(out=q_bf1, in_=q_bf[Lq:128, :])
            S[2 * p]["q_bf"] = q_bf[0:Lq, :]
            S[2 * p + 1]["q_bf"] = q_bf1
            P[p].update(s1b=s1b)

        def stage_A1(b):
            """per-batch: c cast -> cT transposes -> sT matmuls"""
            c_sb = c_tiles[b]
            c_bf = cbpool.tile([128, 4, H], BF16, name="c_bf")
            nc.vector.tensor_copy(out=c_bf[:, 0:2, :], in_=c_sb[:, 0:2, :])
            nc.scalar.activation(out=c_bf[:, 2:4, :], in_=c_sb[:, 2:4, :], func=copy_f)
            # cT[f] = c^T chunk (H rows f*128.., all Lc cols), bf16
            cT = ctpool.tile([128, 4, H], BF16, name="cT")
            for j in range(4):
                pt_c = ps_mm.tile([128, 4, 128], BF16, name="pt_c", tag="big1")
                for f in range(4):
                    nc.tensor.transpose(pt_c[:, f, :], c_bf[:, j, ts(f, 128)], ident)
                nc.vector.tensor_copy(out=cT[:, :, ts(j, 128)], in_=pt_c)

            # sT accumulation: rows 0..63 = qs@cT, rows 64/96 = s0 hi/lo parts
            lhsT = S[b].pop("lhsT")
            ps_sT = ps_mm.tile([128, 512], F32, name="ps_sT", tag="big1")
            for f in range(4):
                nc.tensor.matmul(
                    ps_sT[0:97, :], lhsT[:, f, :], cT[:, f, :],
                    start=(f == 0), stop=False,
                )
            s0hl = spool.tile([128, H], mybir.dt.float32r, name="s0hl")
            nc.scalar.activation(out=s0hl[64:128, :], in_=ps_sT[64:128, :], func=copy_f)
            S[b].update(c_sb=c_sb, c_bf=c_bf, ps_sT=ps_sT, s0hl=s0hl)

        def stage_A2(b):
            """s0 aug matmul -> exp (emitted late so the aug's s0hl wait
            overlaps the next batch's transposes on the PE queue)"""
            p, off = b // 2, (b % 2) * 64
            ps_sT = S[b].pop("ps_sT")
            s0hl = S[b].pop("s0hl")
            nc.tensor.matmul(
                ps_sT[0:97, :], aug[64:128, :], s0hl[64:128, :],
                start=False, stop=True,
            )
            # E = exp(sT + s1 + bias) in bf16; rowsum (f32) for a2
            E_sb = epool.tile([Lq, H], BF16, name="E_sb")
            rowsum = spool.tile([Lq, 1], F32, name="rowsum")
            nc.scalar.activation(
                out=E_sb, in_=ps_sT[0:64, :], func=exp_f,
                bias=P[p]["s1b"][off:off + 64, :], scale=1.0,
                accum_out=rowsum,
            )
            S[b].update(E_sb=E_sb, rowsum=rowsum)

        def stage_B(b):
            """a2 softmax -> a2 transposes -> M2 = a2^T @ c ; colsums"""
            p, off = b // 2, (b % 2) * 64
            c_bf = S[b]["c_bf"]
            E_sb = S[b]["E_sb"]
            ps_S = ps_tr.tile([128, 4], F32, name="ps_S", tag="trq", bufs=1)
            for m in range(4):
                nc.tensor.matmul(
                    ps_S[:, m:m + 1], E_sb[:, ts(m, 128)],
                    ones_col[0:Lq, :], start=True, stop=True,
                )
            rS = spool.tile([128, 4], F32, name="rS")
            nc.vector.reciprocal(rS, ps_S)
            ra2 = spool.tile([Lq, 1], F32, name="ra2")
            nc.vector.reciprocal(ra2, S[b]["rowsum"])
            a2T = epool.tile([Lq, H], BF16, name="a2T")
            nc.vector.tensor_scalar_mul(a2T, E_sb, ra2)

            # a2 natural layout [i, j] via PE transposes of a2T
            a2n = btpool.tile([128, 4, Lq], BF16, name="a2n")
            pt_a = ps_tr.tile([128, 4, 64], BF16, name="pt_a", tag="trq", bufs=1)
            for f in range(4):
                nc.tensor.transpose(pt_a[:, f, :], a2T[:, ts(f, 128)], ident[0:64, 0:64])
            nc.vector.tensor_copy(out=a2n, in_=pt_a)

            # M2 = a2^T @ c  [Lq, H]  (b = a1 @ M2 afterwards - associativity)
            ps_M2 = ps_mm.tile([128, 512], F32, name="ps_M2", tag="big1")
            for jj in range(4):
                nc.tensor.matmul(
                    ps_M2[0:64, :], a2n[:, jj, :], c_bf[:, jj, :],
                    start=(jj == 0), stop=(jj == 3),
                )
            M2_bf = epool.tile([Lq, H], BF16, name="M2_bf")
            nc.scalar.activation(out=M2_bf, in_=ps_M2[0:64, :], func=copy_f)

            S[b].update(rS=rS, M2_bf=M2_bf)

        def stage_C(b, ms):
            """per i-tile: a / b matmuls, scales, products, store"""
            c_sb = S[b]["c_sb"]
            E_sb = S[b]["E_sb"]
            q_bf = S[b]["q_bf"]
            M2_bf = S[b]["M2_bf"]
            rS = S[b]["rS"]
            for m in ms:
                stage = opool.tile([128, 3, H], F32, name="stage")
                ps = ps_ab.tile([128, 2 * H], F32, name="ps", tag="big2")
                nc.tensor.matmul(
                    ps[:, 0:H], E_sb[:, ts(m, 128)], q_bf,
                    start=True, stop=True,
                )
                nc.tensor.matmul(
                    ps[:, H:2 * H], E_sb[:, ts(m, 128)], M2_bf,
                    start=True, stop=True,
                )
                # a = (E^T chunk @ q) * rS ; ca = c * a ; b = (a1@M2)*rS ; cb = c*b
                nc.scalar.activation(out=stage[:, 0, :], in_=ps[:, 0:H], func=copy_f, scale=rS[:, m:m + 1])
                if m % 2 == 0:
                    nc.vector.scalar_tensor_tensor(
                        out=stage[:, 1, :], in0=ps[:, 0:H], scalar=rS[:, m:m + 1],
                        in1=c_sb[:, m, :], op0=MULT, op1=MULT,
                    )
                    nc.vector.scalar_tensor_tensor(
                        out=stage[:, 2, :], in0=ps[:, H:2 * H], scalar=rS[:, m:m + 1],
                        in1=c_sb[:, m, :], op0=MULT, op1=MULT,
                    )
                else:
                    nc.gpsimd.tensor_mul(stage[:, 1, :], stage[:, 0, :], c_sb[:, m, :])
                    if m == 1:
                        nc.vector.scalar_tensor_tensor(
                            out=stage[:, 2, :], in0=ps[:, H:2 * H], scalar=rS[:, m:m + 1],
                            in1=c_sb[:, m, :], op0=MULT, op1=MULT,
                        )
                    else:
                        # b staged by scalar, then multiplied by c in place
                        nc.scalar.activation(out=stage[:, 2, :], in_=ps[:, H:2 * H], func=copy_f, scale=rS[:, m:m + 1])
                        nc.gpsimd.tensor_mul(stage[:, 2, :], stage[:, 2, :], c_sb[:, m, :])
                # store: out tile = [a | c*a | c*b]
                nc.sync.dma_start(out=out_ap[b, ts(m, 128), :], in_=stage)
            if ms[-1] == 3:
                S[b].clear()

        # software-pipelined emission; A(b+1) is queued on PE before B(b) so
        # the PE has transpose work during the exp(b) -> a2T(b) round trip
        stage_PQ(0)
        stage_A1(0)
        stage_A1(1)
        stage_A2(0)
        stage_A2(1)
        stage_B(0)
        stage_C(0, [0, 1])
        stage_B(1)
        stage_C(0, [2, 3])
        stage_PQ(1)
        stage_A1(2)
        stage_C(1, [0, 1])
        stage_A2(2)
        stage_C(1, [2, 3])
        stage_A1(3)
        stage_B(2)
        stage_A2(3)
        stage_C(2, [0, 1])
        stage_B(3)
        stage_C(2, [2, 3])
        stage_C(3, [0, 1])
        stage_C(3, [2, 3])

    nc.compile()
    return nc


def _numpy_fallback(c, q, c_mask, q_mask, c_weight, q_weight, cq_weight, bias):
    NEG_INF = -1e30
    s0 = c @ c_weight
    s1 = (q @ q_weight).transpose(0, 2, 1)
    s2 = np.einsum("bih,bjh->bij", c * cq_weight, q)
    s = s0 + s1 + s2 + bias

    def softmax(x, mask, axis):
        logits = np.where(mask, x, NEG_INF)
        m = logits.max(axis=axis, keepdims=True)
        e = np.exp(logits - m)
        return e / e.sum(axis=axis, keepdims=True)

    a1 = softmax(s, q_mask[:, None, :], 2)
    a2 = softmax(s, c_mask[:, :, None], 1)
    a = np.einsum("bij,bjh->bih", a1, q)
    bb = np.einsum("bik,bjk->bij", a1, a2)
    bb = np.einsum("bij,bjh->bih", bb, c)
    return np.concatenate([c, a, c * a, c * bb], axis=2).astype(np.float32)


def kernel(c, q, c_mask, q_mask, c_weight, q_weight, cq_weight, bias, **_):
    c = np.asarray(c, dtype=np.float32)
    q = np.asarray(q, dtype=np.float32)
    if not (np.all(c_mask) and np.all(q_mask)):
        # masks are all-ones per the problem spec; keep a correct fallback
        return _numpy_fallback(
            c, q, np.asarray(c_mask), np.asarray(q_mask),
            np.asarray(c_weight, np.float32), np.asarray(q_weight, np.float32),
            np.asarray(cq_weight, np.float32), np.asarray(bias, np.float32),
        )

    if "nc" not in _CACHE:
        _CACHE["nc"] = _build_program()
    nc = _CACHE["nc"]

    cqw = np.ascontiguousarray(np.asarray(cq_weight, np.float32).reshape(H))
    cwgt = np.ascontiguousarray(np.asarray(c_weight, np.float32).reshape(H))
    qwgt = np.ascontiguousarray(np.asarray(q_weight, np.float32).reshape(H))
    bias_a = np.ascontiguousarray(np.asarray(bias, np.float32).reshape(1))

    in_maps = []
    for k in range(N_CORES):
        in_maps.append(
            {
                "c": np.ascontiguousarray(c[k * BPC : (k + 1) * BPC]),
                "q": np.ascontiguousarray(q[k * BPC : (k + 1) * BPC]),
                "cqw": cqw,
                "cwgt": cwgt,
                "qwgt": qwgt,
                "bias": bias_a,
            }
        )
    res = run_bass_kernel_spmd(nc, in_maps, core_ids=list(range(N_CORES)))
    out = np.empty((B, Lc, 4 * H), dtype=np.float32)
    out[:, :, 0:H] = c
    for k in range(N_CORES):
        out[k * BPC : (k + 1) * BPC, :, H:] = res.results[k]["out"]
    return out


# revision 39
# speedup vs baseline: 1.0676x; 1.0676x over previous
"""BiDAF attention kernel for 8 Trainium2 NeuronCores.

Data-parallel over batch (B=32 -> 4 per core). Per batch, on-chip:
  sT[j,i] = (q*cqw) @ c^T + s0[i] + s1[j] + bias   (bf16 matmuls, fp32 accum;
  s0 comes free as rows 64/96 of the same matmul via cwgt hi/lo columns, then
  a rank-1 fp32 matmul broadcasts it across rows)
  E = exp(sT)  (one exp serves both softmaxes; s1+bias fused via act bias)
  a2T = E / rowsum(E);  a1 normalization deferred: 1/colsum(E) applied to
  output rows of downstream matmuls.
  a = a1 @ q; b = a1 @ (a2^T @ c); out = [c, a, c*a, c*b]
q-side prep (casts, qs, s1, q^T) is batch-pair-packed to fill 128 partitions.
DMA: input loads dispatched first on the sync queue, c-passthrough stores on
the scalar queue, m-tile stores stream on sync behind compute.
"""

import sys

if "/opt/trn_rl_repo" not in sys.path:
    sys.path.insert(0, "/opt/trn_rl_repo")

from contextlib import ExitStack

import numpy as np

import concourse.bacc as bacc
import concourse.bass as bass
import concourse.mybir as mybir
from concourse.bass import ts
from concourse.bass_utils import run_bass_kernel_spmd
from concourse.masks import make_identity
from concourse.tile import TileContext

N_CORES = 8
B, Lc, Lq, H = 32, 512, 64, 512
BPC = B // N_CORES  # batches per core
F32 = mybir.dt.float32
BF16 = mybir.dt.bfloat16
MULT = mybir.AluOpType.mult

_CACHE = {}


def _build_program():
    nc = bacc.Bacc("TRN2", target_bir_lowering=False, debug=False, num_devices=N_CORES)
    c_h = nc.dram_tensor("c", [BPC, Lc, H], F32, kind="ExternalInput")
    q_h = nc.dram_tensor("q", [BPC, Lq, H], F32, kind="ExternalInput")
    cqw_h = nc.dram_tensor("cqw", [H], F32, kind="ExternalInput")
    cwgt_h = nc.dram_tensor("cwgt", [H], F32, kind="ExternalInput")
    qwgt_h = nc.dram_tensor("qwgt", [H], F32, kind="ExternalInput")
    bias_h = nc.dram_tensor("bias", [1], F32, kind="ExternalInput")
    out_h = nc.dram_tensor("out", [BPC, Lc, 3 * H], F32, kind="ExternalOutput")

    c_ap = c_h.ap()
    q_ap = q_h.ap()
    out_ap = out_h.ap()

    exp_f = mybir.ActivationFunctionType.Exp
    ident_f = mybir.ActivationFunctionType.Identity
    copy_f = mybir.ActivationFunctionType.Copy

    with TileContext(nc) as tc, ExitStack() as ctx:
        const = ctx.enter_context(tc.tile_pool(name="const", bufs=1))
        cpool = ctx.enter_context(tc.tile_pool(name="cpool", bufs=4))
        cbpool = ctx.enter_context(tc.tile_pool(name="cbpool", bufs=3))
        ctpool = ctx.enter_context(tc.tile_pool(name="ctpool", bufs=3))
        qpool = ctx.enter_context(tc.tile_pool(name="qpool", bufs=2))
        spool = ctx.enter_context(tc.tile_pool(name="spool", bufs=3))
        lpool = ctx.enter_context(tc.tile_pool(name="lpool", bufs=3))
        epool = ctx.enter_context(tc.tile_pool(name="epool", bufs=3))
        btpool = ctx.enter_context(tc.tile_pool(name="btpool", bufs=2))
        opool = ctx.enter_context(tc.tile_pool(name="opool", bufs=4))
        ps_tr = ctx.enter_context(tc.tile_pool(name="ps_tr", bufs=1, space="PSUM"))
        ps_mm = ctx.enter_context(tc.tile_pool(name="ps_mm", bufs=3, space="PSUM"))
        ps_ab = ctx.enter_context(tc.tile_pool(name="ps_ab", bufs=2, space="PSUM"))

        NP = BPC // 2  # pairs per core

        # ---- input loads: dispatched first on the sync queue ----
        c_tiles = []
        q_pairs = []
        for p in range(NP):
            b0, b1 = 2 * p, 2 * p + 1
            q_t = qpool.tile([128, H], F32, name="q_sb")
            nc.sync.dma_start(out=q_t[0:Lq, :], in_=q_ap[b0])
            nc.sync.dma_start(out=q_t[Lq:128, :], in_=q_ap[b1])
            c0 = cpool.tile([128, 4, H], F32, name="c_sb")
            nc.sync.dma_start(out=c0, in_=c_ap[b0].rearrange("(j p) h -> p j h", p=128))
            c1 = cpool.tile([128, 4, H], F32, name="c_sb")
            nc.sync.dma_start(out=c1, in_=c_ap[b1].rearrange("(j p) h -> p j h", p=128))
            c_tiles += [c0, c1]
            q_pairs.append(q_t)

        # ---- constants (gpsimd queue; overlap the input loads) ----
        ident = const.tile([128, 128], BF16, name="ident")
        make_identity(nc, ident)
        cw_bc = const.tile([128, H], F32, name="cw_bc")  # cq_weight bcast over rows
        nc.gpsimd.dma_start(out=cw_bc, in_=bass.AP(tensor=cqw_h, offset=0, ap=[[0, 128], [1, H]]))
        qw_bc = const.tile([128, H], F32, name="qw_bc")  # q_weight bcast over rows
        nc.gpsimd.dma_start(out=qw_bc, in_=bass.AP(tensor=qwgt_h, offset=0, ap=[[0, 128], [1, H]]))
        cwgt_col = const.tile([128, 4], F32, name="cwgt_col")  # c_weight as 4 chunks
        nc.gpsimd.dma_start(out=cwgt_col, in_=bass.AP(tensor=cwgt_h, offset=0, ap=[[1, 128], [128, 4]]))
        cwgt_hi = const.tile([128, 4], BF16, name="cwgt_hi")
        nc.vector.tensor_copy(out=cwgt_hi, in_=cwgt_col)
        cwgt_res = const.tile([128, 4], F32, name="cwgt_res")
        nc.vector.tensor_sub(cwgt_res, cwgt_col, cwgt_hi)
        cwgt_lo = const.tile([128, 4], BF16, name="cwgt_lo")
        nc.vector.tensor_copy(out=cwgt_lo, in_=cwgt_res)
        bias_bc = const.tile([128, 1], F32, name="bias_bc")
        nc.gpsimd.dma_start(out=bias_bc, in_=bass.AP(tensor=bias_h, offset=0, ap=[[0, 128], [1, 1]]))
        ones_col = const.tile([128, 1], BF16, name="ones_col")
        nc.vector.memset(ones_col, 1.0)
        aug_f = const.tile([128, 97], F32, name="aug_f")
        nc.vector.memset(aug_f, 0.0)
        nc.vector.memset(aug_f[64:65, 0:64], 1.0)
        nc.vector.memset(aug_f[96:97, 0:64], 1.0)
        aug = const.tile([128, 97], mybir.dt.float32r, name="aug")  # s0 hi+lo add
        nc.vector.tensor_copy(out=aug, in_=aug_f)

        P = [dict() for _ in range(NP)]  # per-pair tile state
        S = [dict() for _ in range(BPC)]  # per-batch tile state

        def stage_PQ(p):
            """pair-level q-side prep: casts, qs, s1, qT transposes, lhsT."""
            q_sb = q_pairs[p]
            q_bf = qpool.tile([128, H], BF16, name="q_bf")
            nc.vector.tensor_copy(out=q_bf, in_=q_sb)
            # qs = q * cq_weight (bf16 out) ; s1 = (q @ q_weight) + bias
            qs_bf = qpool.tile([128, H], BF16, name="qs_bf")
            nc.vector.tensor_mul(qs_bf, q_sb, cw_bc)
            s1_scr = qpool.tile([128, H], F32, name="s1_scr")
            s1_raw = spool.tile([128, 1], F32, name="s1_raw")
            nc.gpsimd.tensor_mul(s1_scr, q_sb, qw_bc)
            nc.vector.tensor_reduce(
                out=s1_raw, in_=s1_scr, axis=mybir.AxisListType.X,
                op=mybir.AluOpType.add,
            )
            s1b = spool.tile([128, 1], F32, name="s1b")
            nc.scalar.activation(out=s1b, in_=s1_raw, func=ident_f, bias=bias_bc, scale=1.0)

            # qT chunks for the pair: pt_q[:, f, 0:64] = b0, [:, f, 64:128] = b1
            qsT = ps_tr.tile([128, 4, 128], BF16, name="qsT", tag="trq", bufs=1)
            for f in range(4):
                nc.tensor.transpose(qsT[:, f, :], qs_bf[:, ts(f, 128)], ident)
            # lhsT[b][f] = [ (qs_b chunk f)^T | cwgt_hi f | junk | cwgt_lo f ]
            # (hi lands in psum row 64, lo in row 96: engine reads need
            # 32-aligned base partitions; rows 65..95 are unused junk)
            for b in (2 * p, 2 * p + 1):
                off = (b % 2) * 64
                lhsT = lpool.tile([128, 4, 97], BF16, name="lhsT")
                # zero cols 65..95: psum rows 65..95 must be finite (the s0
                # aug matmul contracts over them with zero weights)
                nc.vector.memset(lhsT[:, :, 65:96], 0.0)
                nc.vector.tensor_copy(out=lhsT[:, :, 0:64], in_=qsT[:, :, off:off + 64])
                nc.vector.tensor_copy(out=lhsT[:, :, 64:65], in_=cwgt_hi.rearrange("p (f o) -> p f o", o=1))
                nc.vector.tensor_copy(out=lhsT[:, :, 96:97], in_=cwgt_lo.rearrange("p (f o) -> p f o", o=1))
                S[b]["lhsT"] = lhsT
            # odd batch's q half must sit at base partition 0 for the a-matmul
            # (PE requires lhsT/rhs at the same base partition); DMA it down.
            q_bf1 = qpool.tile([Lq, H], BF16, name="q_bf1")
            nc.scalar.dma_start(out=q_bf1, in_=q_bf[Lq:128, :])
            S[2 * p]["q_bf"] = q_bf[0:Lq, :]
            S[2 * p + 1]["q_bf"] = q_bf1
            P[p].update(s1b=s1b)

        def stage_A1(b):
            """per-batch: c cast -> cT transposes -> sT matmuls"""
            c_sb = c_tiles[b]
            c_bf = cbpool.tile([128, 4, H], BF16, name="c_bf")
            nc.vector.tensor_copy(out=c_bf[:, 0:2, :], in_=c_sb[:, 0:2, :])
            nc.scalar.activation(out=c_bf[:, 2:4, :], in_=c_sb[:, 2:4, :], func=copy_f)
            # cT[f] = c^T chunk (H rows f*128.., all Lc cols), bf16
            cT = ctpool.tile([128, 4, H], BF16, name="cT")
            for j in range(4):
                pt_c = ps_mm.tile([128, 4, 128], BF16, name="pt_c", tag="big1")
                for f in range(4):
                    nc.tensor.transpose(pt_c[:, f, :], c_bf[:, j, ts(f, 128)], ident)
                nc.vector.tensor_copy(out=cT[:, :, ts(j, 128)], in_=pt_c)

            # sT accumulation: rows 0..63 = qs@cT, rows 64/96 = s0 hi/lo parts
            lhsT = S[b].pop("lhsT")
            ps_sT = ps_mm.tile([128, 512], F32, name="ps_sT", tag="big1")
            for f in range(4):
                nc.tensor.matmul(
                    ps_sT[0:97, :], lhsT[:, f, :], cT[:, f, :],
                    start=(f == 0), stop=False,
                )
            s0hl = spool.tile([128, H], mybir.dt.float32r, name="s0hl")
            nc.scalar.activation(out=s0hl[64:128, :], in_=ps_sT[64:128, :], func=copy_f)
            S[b].update(c_sb=c_sb, c_bf=c_bf, ps_sT=ps_sT, s0hl=s0hl)

        def stage_A2(b):
            """s0 aug matmul -> exp (emitted late so the aug's s0hl wait
            overlaps the next batch's transposes on the PE queue)"""
            p, off = b // 2, (b % 2) * 64
            ps_sT = S[b].pop("ps_sT")
            s0hl = S[b].pop("s0hl")
            nc.tensor.matmul(
                ps_sT[0:97, :], aug[64:128, :], s0hl[64:128, :],
                start=False, stop=True,
            )
            # E = exp(sT + s1 + bias) in bf16; rowsum (f32) for a2
            E_sb = epool.tile([Lq, H], BF16, name="E_sb")
            rowsum = spool.tile([Lq, 1], F32, name="rowsum")
            nc.scalar.activation(
                out=E_sb, in_=ps_sT[0:64, :], func=exp_f,
                bias=P[p]["s1b"][off:off + 64, :], scale=1.0,
                accum_out=rowsum,
            )
            S[b].update(E_sb=E_sb, rowsum=rowsum)

        def stage_B(b):
            """a2 softmax -> a2 transposes -> M2 = a2^T @ c ; colsums"""
            p, off = b // 2, (b % 2) * 64
            c_bf = S[b]["c_bf"]
            E_sb = S[b]["E_sb"]
            ra2 = spool.tile([Lq, 1], F32, name="ra2")
            nc.vector.reciprocal(ra2, S[b]["rowsum"])
            a2T = epool.tile([Lq, H], BF16, name="a2T")
            nc.vector.tensor_scalar_mul(a2T, E_sb, ra2)

            # a2 natural layout [i, j] via PE transposes of a2T
            a2n = btpool.tile([128, 4, Lq], BF16, name="a2n")
            pt_a = ps_tr.tile([128, 4, 64], BF16, name="pt_a", tag="trq", bufs=1)
            for f in range(4):
                nc.tensor.transpose(pt_a[:, f, :], a2T[:, ts(f, 128)], ident[0:64, 0:64])
            nc.vector.tensor_copy(out=a2n, in_=pt_a)

            # M2 = a2^T @ c  [Lq, H]  (b = a1 @ M2 afterwards - associativity)
            ps_M2 = ps_mm.tile([128, 512], F32, name="ps_M2", tag="big1")
            for jj in range(4):
                nc.tensor.matmul(
                    ps_M2[0:64, :], a2n[:, jj, :], c_bf[:, jj, :],
                    start=(jj == 0), stop=(jj == 3),
                )
            M2_bf = epool.tile([Lq, H], BF16, name="M2_bf")
            nc.vector.tensor_copy(out=M2_bf, in_=ps_M2[0:64, :])

            # column sums of E (normalizer of a1), reciprocal per i-tile
            ps_S = ps_tr.tile([128, 4], F32, name="ps_S", tag="trq", bufs=1)
            for m in range(4):
                nc.tensor.matmul(
                    ps_S[:, m:m + 1], E_sb[:, ts(m, 128)],
                    ones_col[0:Lq, :], start=True, stop=True,
                )
            rS = spool.tile([128, 4], F32, name="rS")
            nc.vector.reciprocal(rS, ps_S)
            S[b].update(rS=rS, M2_bf=M2_bf)

        def stage_C(b, ms):
            """per i-tile: a / b matmuls, scales, products, store"""
            c_sb = S[b]["c_sb"]
            E_sb = S[b]["E_sb"]
            q_bf = S[b]["q_bf"]
            M2_bf = S[b]["M2_bf"]
            rS = S[b]["rS"]
            for m in ms:
                stage = opool.tile([128, 3, H], F32, name="stage")
                ps = ps_ab.tile([128, 2 * H], F32, name="ps", tag="big2")
                nc.tensor.matmul(
                    ps[:, 0:H], E_sb[:, ts(m, 128)], q_bf,
                    start=True, stop=True,
                )
                nc.tensor.matmul(
                    ps[:, H:2 * H], E_sb[:, ts(m, 128)], M2_bf,
                    start=True, stop=True,
                )
                # a = (E^T chunk @ q) * rS ; ca = c * a ; b = (a1@M2)*rS ; cb = c*b
                nc.scalar.activation(out=stage[:, 0, :], in_=ps[:, 0:H], func=copy_f, scale=rS[:, m:m + 1])
                if m % 2 == 0:
                    nc.vector.scalar_tensor_tensor(
                        out=stage[:, 1, :], in0=ps[:, 0:H], scalar=rS[:, m:m + 1],
                        in1=c_sb[:, m, :], op0=MULT, op1=MULT,
                    )
                    nc.vector.scalar_tensor_tensor(
                        out=stage[:, 2, :], in0=ps[:, H:2 * H], scalar=rS[:, m:m + 1],
                        in1=c_sb[:, m, :], op0=MULT, op1=MULT,
                    )
                else:
                    nc.gpsimd.tensor_mul(stage[:, 1, :], stage[:, 0, :], c_sb[:, m, :])
                    if m == 1:
                        nc.vector.scalar_tensor_tensor(
                            out=stage[:, 2, :], in0=ps[:, H:2 * H], scalar=rS[:, m:m + 1],
                            in1=c_sb[:, m, :], op0=MULT, op1=MULT,
                        )
                    else:
                        # b staged by scalar, then multiplied by c in place
                        nc.scalar.activation(out=stage[:, 2, :], in_=ps[:, H:2 * H], func=copy_f, scale=rS[:, m:m + 1])
                        nc.gpsimd.tensor_mul(stage[:, 2, :], stage[:, 2, :], c_sb[:, m, :])
                # store: out tile = [a | c*a | c*b]
                nc.sync.dma_start(out=out_ap[b, ts(m, 128), :], in_=stage)
            if ms[-1] == 3:
                S[b].clear()

        # software-pipelined emission; A(b+1) is queued on PE before B(b) so
        # the PE has transpose work during the exp(b) -> a2T(b) round trip
        stage_PQ(0)
        stage_A1(0)
        stage_A1(1)
        stage_A2(0)
        stage_A2(1)
        stage_B(0)
        stage_C(0, [0, 1])
        stage_B(1)
        stage_C(0, [2, 3])
        stage_PQ(1)
        stage_A1(2)
        stage_C(1, [0, 1])
        stage_A2(2)
        stage_C(1, [2, 3])
        stage_A1(3)
        stage_B(2)
        stage_A2(3)
        stage_C(2, [0, 1])
        stage_B(3)
        stage_C(2, [2, 3])
        stage_C(3, [0, 1])
        stage_C(3, [2, 3])

    nc.compile()
    return nc


def _numpy_fallback(c, q, c_mask, q_mask, c_weight, q_weight, cq_weight, bias):
    NEG_INF = -1e30
    s0 = c @ c_weight
    s1 = (q @ q_weight).transpose(0, 2, 1)
    s2 = np.einsum("bih,bjh->bij", c * cq_weight, q)
    s = s0 + s1 + s2 + bias

    def softmax(x, mask, axis):
        logits = np.where(mask, x, NEG_INF)
        m = logits.max(axis=axis, keepdims=True)
        e = np.exp(logits - m)
        return e / e.sum(axis=axis, keepdims=True)

    a1 = softmax(s, q_mask[:, None, :], 2)
    a2 = softmax(s, c_mask[:, :, None], 1)
    a = np.einsum("bij,bjh->bih", a1, q)
    bb = np.einsum("bik,bjk->bij", a1, a2)
    bb = np.einsum("bij,bjh->bih", bb, c)
    return np.concatenate([c, a, c * a, c * bb], axis=2).astype(np.float32)


def kernel(c, q, c_mask, q_mask, c_weight, q_weight, cq_weight, bias, **_):
    c = np.asarray(c, dtype=np.float32)
    q = np.asarray(q, dtype=np.float32)
    if not (np.all(c_mask) and np.all(q_mask)):
        # masks are all-ones per the problem spec; keep a correct fallback
        return _numpy_fallback(
            c, q, np.asarray(c_mask), np.asarray(q_mask),
            np.asarray(c_weight, np.float32), np.asarray(q_weight, np.float32),
            np.asarray(cq_weight, np.float32), np.asarray(bias, np.float32),
        )

    if "nc" not in _CACHE:
        _CACHE["nc"] = _build_program()
    nc = _CACHE["nc"]

    cqw = np.ascontiguousarray(np.asarray(cq_weight, np.float32).reshape(H))
    cwgt = np.ascontiguousarray(np.asarray(c_weight, np.float32).reshape(H))
    qwgt = np.ascontiguousarray(np.asarray(q_weight, np.float32).reshape(H))
    bias_a = np.ascontiguousarray(np.asarray(bias, np.float32).reshape(1))

    in_maps = []
    for k in range(N_CORES):
        in_maps.append(
            {
                "c": np.ascontiguousarray(c[k * BPC : (k + 1) * BPC]),
                "q": np.ascontiguousarray(q[k * BPC : (k + 1) * BPC]),
                "cqw": cqw,
                "cwgt": cwgt,
                "qwgt": qwgt,
                "bias": bias_a,
            }
        )
    res = run_bass_kernel_spmd(nc, in_maps, core_ids=list(range(N_CORES)))
    out = np.empty((B, Lc, 4 * H), dtype=np.float32)
    out[:, :, 0:H] = c
    for k in range(N_CORES):
        out[k * BPC : (k + 1) * BPC, :, H:] = res.results[k]["out"]
    return out
